# revision 7
# baseline (speedup 1.0000x reference)
"""Multi-head attention (B=2, M=N=2048, D=1024, H=16, DH=64) on 8 TRN2 cores.

Sharding: data-parallel over batch (cores 0-3 = batch 0, 4-7 = batch 1),
tensor-parallel over heads within each batch group (4 heads/core). All
f32 matmuls run in f32r (tf32-class PE fast path); V/attention-weights
run in bf16 (errors average under softmax).

Per core (v2 — PE-lean attention inner loop):
  - stage 1, chunked per 512 seq rows: PE-transpose x into
    chan-on-partition layout (f32r transposes: 1.5 cyc/row vs 2.0 for
    f32), then project. K^T/Q^T land pair-packed ([64j:64j+64] of pair
    tile p = head 2p+j) for row-tiled S^T; V lands natural [n, head
    chans] in bf16 with NO ones/zeros padding (v1 wasted half the AV
    matmul columns on it).
  - stage 2, per m-chunk of 512 query rows, per n-tile: 2x row-tiled
    K=64 S^T matmuls per pair (disjoint PE row groups, concurrent);
    ScalarE Exp (PSUM f32 -> SBUF bf16, ScalarE does NOTHING else in
    this phase); AV via 2x column-tiled matmuls (head 2p -> PSUM
    partitions 0:64 at tile (0,0), head 2p+1 -> 64:128 at (0,64)) so a
    pair costs one 512-cycle stream instead of two; softmax denominators
    via a separate 4-way column-tiled pass (ones lhsT, head h -> PSUM
    partition 32h at tile (0,32h)) accumulated across n-tiles.
    Normalization: DVE reciprocal directly off the denominator PSUM row,
    gpsimd partition-broadcast, DVE multiplies. Per-m AllGather of the
    (256, 512) O^T shard across the 4-core batch group.
  - Q^T prep for m-chunk m+1 is EMITTED between stage 2(m) and stage
    2(m+1): the exp-paced attention loop leaves PE gaps that the
    transposes/projections of the next chunk fill.
  - stage 3 (emitted last => lowest PE priority): 256-wide
    output-channel slice out^T = Wo_slice^T.T @ O^T_full per m-chunk
    from the gathered shards; bias added on DVE (not ScalarE).
Host-side prep as v1: weights pre-transposed/sliced per core, bv folded
into bo_eff = bo + Wo @ bv, bk dropped (cancels in softmax).
"""

import os

import numpy as np

B, M, NSEQ, D = 2, 2048, 2048, 1024
H, DH = 16, 64
HC = 4                # heads per core
PC = HC * DH          # 256 projected channels per core
CT = D // 128         # 8 contraction tiles
NT = NSEQ // 128      # 16 n-tiles
MT = M // 512         # 4 m-chunks
NCORES = 8

_CACHE = {}


def _build(single_core=False, reps=1):
    import concourse.bass as bass
    import concourse.tile as tile
    from concourse import bacc, mybir
    from concourse.masks import make_identity

    F32 = mybir.dt.float32
    F32R = mybir.dt.float32r
    BF16 = mybir.dt.bfloat16
    AF = mybir.ActivationFunctionType

    nc = bacc.Bacc(
        "TRN2",
        target_bir_lowering=False,
        debug=False,
        num_devices=1 if single_core else 8,
    )

    xq_d = nc.dram_tensor("xq", [M, D], F32R, kind="ExternalInput")
    xk_d = nc.dram_tensor("xk", [NSEQ, D], F32R, kind="ExternalInput")
    xv_d = nc.dram_tensor("xv", [NSEQ, D], F32R, kind="ExternalInput")
    wqT_d = nc.dram_tensor("wqT", [D, PC], F32R, kind="ExternalInput")
    wkT_d = nc.dram_tensor("wkT", [D, PC], F32R, kind="ExternalInput")
    wvT_d = nc.dram_tensor("wvT", [D, PC], F32R, kind="ExternalInput")
    woT_d = nc.dram_tensor("woT", [D, PC], F32R, kind="ExternalInput")
    bq_d = nc.dram_tensor("bq", [PC, 1], F32, kind="ExternalInput")
    bo_d = nc.dram_tensor("bo", [PC, 1], F32, kind="ExternalInput")
    outT_d = nc.dram_tensor("outT", [PC, M], F32, kind="ExternalOutput")
    debug = bool(int(os.environ.get("KERNEL_DEBUG", "0")))
    if debug:
        qT_dbg = nc.dram_tensor("qT_dbg", [128, 2, M], F32, kind="ExternalOutput")
        kT_dbg = nc.dram_tensor("kT_dbg", [128, 2, NSEQ], F32, kind="ExternalOutput")
        v_dbg = nc.dram_tensor("v_dbg", [128, HC, NT, DH], F32, kind="ExternalOutput")
        agin_dbg = nc.dram_tensor("agin_dbg", [PC, M], F32, kind="ExternalOutput")
        agout_dbg = nc.dram_tensor(
            "agout_dbg", [4 * PC, M], F32, kind="ExternalOutput"
        )

    with tile.TileContext(nc) as tc:
        with (
            tc.tile_pool(name="singles", bufs=1) as singles,
            tc.tile_pool(name="dram", bufs=1, space="DRAM") as dram,
        ):
            ident_f32 = singles.tile([128, 128], F32)
            make_identity(nc, ident_f32)
            ident = singles.tile([128, 128], F32R)
            nc.vector.tensor_copy(ident, ident_f32)
            bq_sb = singles.tile([128, 2], F32)
            nc.sync.dma_start(
                out=bq_sb, in_=bq_d[:, :].rearrange("(o p) w -> p (o w)", p=128)
            )
            bo_sb = singles.tile([128, 2], F32)
            nc.sync.dma_start(
                out=bo_sb, in_=bo_d[:, :].rearrange("(o p) w -> p (o w)", p=128)
            )
            ones1 = singles.tile([128, 1], BF16)
            nc.vector.memset(ones1, 1.0)

            ag_in = dram.tile([MT, PC, 512], F32R)
            ag_out = dram.tile([MT, 4 * PC, 512], F32R)

            for rep in range(reps):
                _emit_rep(
                    nc, tc, bass, mybir, F32, F32R, BF16, AF, rep, single_core,
                    debug and rep == reps - 1,
                    dict(
                        xq_d=xq_d, xk_d=xk_d, xv_d=xv_d, wqT_d=wqT_d,
                        wkT_d=wkT_d, wvT_d=wvT_d, woT_d=woT_d, outT_d=outT_d,
                        ident=ident, bq_sb=bq_sb, bo_sb=bo_sb, ones1=ones1,
                        ag_in=ag_in, ag_out=ag_out,
                        dbg=dict(
                            qT_dbg=qT_dbg, kT_dbg=kT_dbg, v_dbg=v_dbg,
                            agin_dbg=agin_dbg, agout_dbg=agout_dbg,
                        ) if debug else None,
                    ),
                )
    nc.compile()
    return nc


def _emit_rep(nc, tc, bass, mybir, F32, F32R, BF16, AF, rep, single_core,
              debug, env):
    ident = env["ident"]
    bq_sb, bo_sb, ones1 = env["bq_sb"], env["bo_sb"], env["ones1"]
    ag_in, ag_out = env["ag_in"], env["ag_out"]
    R = f"r{rep}_"

    with (
        tc.tile_pool(name=f"{R}proj_out", bufs=1) as proj_out,
        tc.tile_pool(name=f"{R}nat", bufs=6) as nat_pool,
        tc.tile_pool(name=f"{R}xtc", bufs=2) as xtc_pool,
        tc.tile_pool(name=f"{R}wsb", bufs=2) as w_pool,
        tc.tile_pool(name=f"{R}wo", bufs=1) as wo_pool,
        tc.tile_pool(name=f"{R}at", bufs=6) as at_pool,
        tc.tile_pool(name=f"{R}eps", bufs=4) as eps_pool,
        tc.tile_pool(name=f"{R}osc", bufs=3) as osc_pool,
        tc.tile_pool(name=f"{R}og", bufs=2) as og_pool,
        tc.tile_pool(name=f"{R}osb", bufs=3) as osb_pool,
        tc.tile_pool(name=f"{R}ps_ga", bufs=2, space="PSUM") as ps_ga,
        tc.tile_pool(name=f"{R}ps_ss", bufs=3, space="PSUM") as ps_ss,
        tc.tile_pool(name=f"{R}ps_av", bufs=2, space="PSUM") as ps_av,
        tc.tile_pool(name=f"{R}ps_dn", bufs=1, space="PSUM") as ps_dn,
    ):
        qT = proj_out.tile([128, 2, M], F32R, name=f"{R}qT")  # [part, pair, m]
        kT = proj_out.tile([128, 2, NSEQ], F32R, name=f"{R}kT")
        v_sb = proj_out.tile([128, HC, NT, DH], BF16, name=f"{R}v_sb")

        wk_sb = w_pool.tile([128, CT, PC], F32R, tag="w", name=f"{R}wk")
        nc.gpsimd.dma_start(
            out=wk_sb, in_=env["wkT_d"][:, :].rearrange("(ct p) c -> p ct c", p=128)
        )
        wv_sb = w_pool.tile([128, CT, PC], F32R, tag="w", name=f"{R}wv")
        nc.gpsimd.dma_start(
            out=wv_sb, in_=env["wvT_d"][:, :].rearrange("(ct p) c -> p ct c", p=128)
        )
        wo_sb = wo_pool.tile([128, CT, PC], F32R, name=f"{R}wo_sb")
        nc.gpsimd.dma_start(
            out=wo_sb, in_=env["woT_d"][:, :].rearrange("(ct p) c -> p ct c", p=128)
        )

        def _scalar_copy(dst, src):
            nc.scalar.activation(dst, src, AF.Copy)

        def copy_engine(i, allow_scalar):
            # round-robin PSUM evacuation: DVE/ScalarE only (GPSIMD cannot
            # read PSUM); ScalarE only outside exp phases
            if allow_scalar:
                return (nc.vector.tensor_copy, _scalar_copy)[i % 2]
            return nc.vector.tensor_copy

        def prep_chunk(x_d, w_sb, c, kind, dst, allow_scalar, tagp):
            """Transpose 512 seq rows of x and project them.

            kind: 'kq' -> dst[:, ot, c*512:(c+1)*512] gets W^T.T @ x^T
                  (chan-on-partition); 'v' -> dst[:, :, c*4+i, :] gets
                  x @ W^T rows (seq-on-partition), bf16.
            """
            nats = []
            for i in range(4):
                nt_t = nat_pool.tile(
                    [128, D], F32R, tag="nat", name=f"{R}nat{tagp}_{c}_{i}"
                )
                r0 = (c * 4 + i) * 128
                nc.sync.dma_start(out=nt_t, in_=x_d[r0 : r0 + 128, :])
                nats.append(nt_t)
            xtc = xtc_pool.tile(
                [128, CT, 512], F32R, tag="xtc", name=f"{R}xtc{tagp}_{c}"
            )
            for ct in range(CT):
                pst = ps_ga.tile(
                    [128, 512], F32R, tag="ga", name=f"{R}pst{tagp}_{c}_{ct}"
                )
                for i in range(4):
                    nc.tensor.transpose(
                        pst[:, i * 128 : (i + 1) * 128],
                        nats[i][:, ct * 128 : (ct + 1) * 128],
                        ident,
                    )
                copy_engine(ct, allow_scalar)(xtc[:, ct, :], pst)
            if kind == "kq":
                has_bias = dst is qT
                for ot in range(2):
                    pj = ps_ga.tile(
                        [128, 512], F32, tag="ga", name=f"{R}pj{tagp}_{c}_{ot}"
                    )
                    for ct in range(CT):
                        nc.tensor.matmul(
                            pj,
                            w_sb[:, ct, ot * 128 : (ot + 1) * 128],
                            xtc[:, ct, :],
                            start=(ct == 0),
                            stop=(ct == CT - 1),
                        )
                    d = dst[:, ot, c * 512 : (c + 1) * 512]
                    if has_bias:
                        # bk dropped for K; bq kept for Q
                        nc.vector.tensor_scalar_add(d, pj, bq_sb[:, ot : ot + 1])
                    else:
                        copy_engine(ot, allow_scalar)(d, pj)
            else:  # V natural
                for i in range(4):
                    ntile = c * 4 + i
                    psv = ps_ga.tile(
                        [128, 512], F32, tag="ga", name=f"{R}psv{tagp}_{c}_{i}"
                    )
                    for ct in range(CT):
                        nc.tensor.matmul(
                            psv[:, 0:PC],
                            xtc[:, ct, i * 128 : (i + 1) * 128],
                            w_sb[:, ct, :],
                            start=(ct == 0),
                            stop=(ct == CT - 1),
                        )
                    copy_engine(i, allow_scalar)(
                        dst[:, :, ntile, :],
                        psv[:, 0:PC].rearrange("p (h d) -> p h d", h=HC),
                    )

        # ---------------- stage 1: K, V fully; Q chunk 0 ----------------
        for c in range(4):
            prep_chunk(env["xk_d"], wk_sb, c, "kq", kT, True, "k")
        for c in range(4):
            prep_chunk(env["xv_d"], wv_sb, c, "v", v_sb, True, "v")
        wq_sb = w_pool.tile([128, CT, PC], F32R, tag="w", name=f"{R}wq")
        nc.gpsimd.dma_start(
            out=wq_sb, in_=env["wqT_d"][:, :].rearrange("(ct p) c -> p ct c", p=128)
        )
        prep_chunk(env["xq_d"], wq_sb, 0, "kq", qT, True, "q")

        # ------- stage 2: attention, with Q chunk m+1 emitted between ms -------
        for m in range(MT):
            dn = ps_dn.tile([128, 512], F32, tag="dn", name=f"{R}dn{m}")
            avp = [
                ps_av.tile([128, 512], F32, tag="av", name=f"{R}av{m}_{p}")
                for p in range(2)
            ]
            for nt in range(NT):
                ats = []
                for p in range(2):
                    for j in range(2):
                        ss = ps_ss.tile(
                            [128, 512], F32, tag="ss",
                            name=f"{R}ss{m}_{nt}_{p}_{j}",
                        )
                        base = j * 64
                        nc.tensor.matmul(
                            ss,
                            kT[base : base + 64, p, nt * 128 : (nt + 1) * 128],
                            qT[base : base + 64, p, m * 512 : (m + 1) * 512],
                            start=True,
                            stop=True,
                        )
                        at = at_pool.tile(
                            [128, 512], BF16, tag="at",
                            name=f"{R}at{m}_{nt}_{p}_{j}",
                        )
                        nc.scalar.activation(at, ss, AF.Exp)
                        ats.append(at)
                for p in range(2):
                    for j in range(2):
                        h = 2 * p + j
                        nc.tensor.matmul(
                            avp[p][j * 64 : (j + 1) * 64, :],
                            v_sb[:, h, nt, :],
                            ats[h],
                            start=(nt == 0),
                            stop=(nt == NT - 1),
                        )
                for h in range(HC):
                    nc.tensor.matmul(
                        dn[32 * h : 32 * h + 1, :],
                        ones1,
                        ats[h],
                        start=(nt == 0),
                        stop=(nt == NT - 1),
                        tile_position=(0, 32 * h),
                    )
            for p in range(2):
                rbcs = []
                for j in range(2):
                    h = 2 * p + j
                    rec = eps_pool.tile(
                        [1, 512], F32, tag="rec", name=f"{R}rec{m}_{h}"
                    )
                    nc.vector.reciprocal(rec, dn[32 * h : 32 * h + 1, :])
                    rbc = eps_pool.tile(
                        [128, 512], F32, tag="rbc", name=f"{R}rbc{m}_{h}"
                    )
                    nc.gpsimd.partition_broadcast(rbc, rec)
                    rbcs.append(rbc)
                osc = osc_pool.tile(
                    [128, 512], F32R, tag="osc", name=f"{R}osc{m}_{p}"
                )
                nc.vector.tensor_mul(
                    osc[0:64, :], avp[p][0:64, :], rbcs[0][0:64, :]
                )
                nc.vector.tensor_mul(
                    osc[64:128, :], avp[p][64:128, :], rbcs[1][64:128, :]
                )
                nc.sync.dma_start(
                    out=ag_in[m, p * 128 : (p + 1) * 128, :], in_=osc
                )

            if single_core:
                for rr in range(4):
                    nc.sync.dma_start(
                        out=ag_out[m, rr * PC : (rr + 1) * PC, :],
                        in_=ag_in[m, :, :],
                    )
            else:
                nc.gpsimd.collective_compute(
                    "AllGather",
                    bass.mybir.AluOpType.bypass,
                    replica_groups=[[0, 1, 2, 3], [4, 5, 6, 7]],
                    ins=[ag_in[m, :, :].opt()],
                    outs=[ag_out[m, :, :].opt()],
                )

            if m + 1 < MT:
                # Q^T prep for the next m-chunk fills this chunk's PE gaps
                prep_chunk(env["xq_d"], wq_sb, m + 1, "kq", qT, False, "q")

        # stage 3: output projection, emitted last => lowest PE priority;
        # its matmuls fill PE gaps instead of delaying the S^T/exp stream.
        for m in range(MT):
            og = og_pool.tile([128, CT, 512], F32R, tag="og", name=f"{R}og{m}")
            for ct in range(CT):
                nc.sync.dma_start(
                    out=og[:, ct, :],
                    in_=ag_out[m, ct * 128 : (ct + 1) * 128, :],
                )
            for ot in range(2):
                po = ps_ga.tile(
                    [128, 512], F32, tag="ga", name=f"{R}po{m}_{ot}"
                )
                for ct in range(CT):
                    nc.tensor.matmul(
                        po,
                        wo_sb[:, ct, ot * 128 : (ot + 1) * 128],
                        og[:, ct, :],
                        start=(ct == 0),
                        stop=(ct == CT - 1),
                    )
                osb = osb_pool.tile(
                    [128, 512], F32, tag="osb", name=f"{R}osb{m}_{ot}"
                )
                nc.vector.tensor_scalar_add(osb, po, bo_sb[:, ot : ot + 1])
                nc.sync.dma_start(
                    out=env["outT_d"][
                        ot * 128 : (ot + 1) * 128, m * 512 : (m + 1) * 512
                    ],
                    in_=osb,
                )

        if debug:
            d = env["dbg"]
            nc.sync.dma_start(out=d["qT_dbg"][:, :, :], in_=qT[:, :, :].bitcast(F32))
            nc.sync.dma_start(out=d["kT_dbg"][:, :, :], in_=kT[:, :, :].bitcast(F32))
            vf = eps_pool.tile([128, HC * NT * DH], F32, tag="vdbg")
            nc.vector.tensor_copy(
                vf[:, :].rearrange("p (h n d) -> p h n d", h=HC, n=NT),
                v_sb[:, :, :, :],
            )
            nc.sync.dma_start(
                out=d["v_dbg"][:, :, :, :],
                in_=vf[:, :].rearrange("p (h n d) -> p h n d", h=HC, n=NT),
            )
            nc.gpsimd.dma_start(
                out=d["agin_dbg"][:, :], in_=ag_in[:, :, :].bitcast(F32)
            )
            nc.gpsimd.dma_start(
                out=d["agout_dbg"][:, :], in_=ag_out[:, :, :].bitcast(F32)
            )


def _make_in_maps(queries, keys, values, Wq, bq, Wk, bk, Wv, bv, Wo, bo):
    # bv folds through attention (softmax weights sum to 1) and the output
    # projection into an effective output bias; bk shifts every logit in a
    # row equally so softmax cancels it.
    bo_eff = bo + Wo @ bv
    c = np.ascontiguousarray
    in_maps = []
    for core in range(NCORES):
        b, r = core // 4, core % 4
        sl = slice(r * PC, (r + 1) * PC)
        in_maps.append(
            {
                "xq": c(queries[b]),
                "xk": c(keys[b]),
                "xv": c(values[b]),
                "wqT": c(Wq[sl, :].T),
                "wkT": c(Wk[sl, :].T),
                "wvT": c(Wv[sl, :].T),
                "woT": c(Wo.T[:, sl]),
                "bq": c(bq[sl].reshape(PC, 1)),
                "bo": c(bo_eff[sl].reshape(PC, 1)),
            }
        )
    return in_maps


def kernel(queries, keys, values, Wq, bq, Wk, bk, Wv, bv, Wo, bo, _trace=False):
    import concourse.bass_utils as bass_utils

    args = [queries, keys, values, Wq, bq, Wk, bk, Wv, bv, Wo, bo]
    args = [np.asarray(a, dtype=np.float32) for a in args]

    if "nc" not in _CACHE:
        _CACHE["nc"] = _build()
    nc = _CACHE["nc"]

    in_maps = _make_in_maps(*args)
    res = bass_utils.run_bass_kernel_spmd(
        nc, in_maps, core_ids=list(range(NCORES)), trace=_trace
    )
    _CACHE["last_result"] = res

    out = np.empty((B, M, D), dtype=np.float32)
    for core in range(NCORES):
        b, r = core // 4, core % 4
        out[b, :, r * PC : (r + 1) * PC] = res.results[core]["outT"].T
    return out
